# revision 19
# baseline (speedup 1.0000x reference)
"""Trainium2 Bass kernel for nn_ConvSPE (two depthwise convs K=201 over z).

Strategy
--------
out[t, c] = sum_j w[j, c] * z[201 + t + j, c]   (t in [0, 2048), per realization r)

Mapped to dense PE matmuls via banded-Toeplitz weight blocks: for output tile
t = 128*T + i, the contraction (i + j) splits into 3 chunks of 128 (m = 0..2):

    out[128T + i] = sum_m sum_p  W_m[p, i] * z[201 + 128(T+m) + p]
    W_m[p, i] = w[128m + p - i]   (zero outside [0, 201))

W_m is independent of T and r, so one stationary [128, 128] weight block
streams all 64 realizations x all 16 output tiles as matmul columns.

Sharding: channels across the 8 cores (64 ch = one head per core); weights and
z-slices per channel are core-private, realizations all stay on-core.

dtypes: fp16 matmul inputs (rel err ~3e-4) accumulated in f32 PSUM; outputs
quantized to fp8 e3m4 with a x16 pre-scale folded into the weights (rel err
~1.3%, inside the 2e-2 budget) to halve output HBM traffic.

DMA layout notes: wt is shipped as [conv][p][c][m][i] so each (group, conv)
weight DMA reads 6 KiB contiguous runs per partition (128 descriptors); the
z DMA ships [c][p][k*64+r] with 2.25 KiB runs; out DMA writes (c, r)-contiguous
512 B runs per (tile, partition).
"""

import numpy as np
import concourse.bass as bass
import concourse.mybir as mybir
from concourse.tile import TileContext
from concourse.bass_utils import run_bass_kernel_spmd

# Problem constants (hardcoded per the task contract)
R = 64
S = 2048
K = 201
C = 512
H = 8
F = 64
PAD_LEN = 4 * K + S  # 2852
SCALE = float((R * F) ** 0.25)  # 8.0
OUT_PRESCALE = 16.0  # folded into weights; host divides it back out

NCORES = 8
CPC = C // NCORES      # 64 channels per core
NK = 18                # 128-element z chunks per channel: u in [201, 201 + 18*128)
NT = S // 128          # 16 output tiles
NM = 3                 # Toeplitz chunks per output tile
GROUP = 8              # channels processed per DMA group
NGROUPS = CPC // GROUP


def _split_sync_waits(nc) -> None:
    """Walrus in this container accepts at most ONE sync wait per instruction.

    Move extra on_wait entries onto same-engine InstNoOp carriers inserted
    immediately before the over-limit instruction (program order on the same
    engine preserves semantics)."""
    ctr = 0
    for f in nc.m.functions:
        for blk in f.blocks:
            new = []
            for inst in blk.instructions:
                si = inst.sync_info
                waits = list(si.on_wait) if (si is not None and si.on_wait) else []
                if len(waits) > 1:
                    for wjob in waits[:-1]:
                        nop = mybir.InstNoOp(name=f"antwaitnop{ctr}", ins=[], outs=[])
                        ctr += 1
                        nop.engine = inst.engine
                        nop.sync_info = mybir.SyncInfo(on_wait=[wjob], on_update=[])
                        new.append(nop)
                    si.on_wait = [waits[-1]]
                new.append(inst)
            blk.instructions = new


def _build_nc():
    """Build the per-core Bass program (identical on all 8 cores)."""
    nc = bass.Bass()
    f32 = mybir.dt.float32
    f16 = mybir.dt.float16
    f8 = mybir.dt.float8e3

    # zt: [CPC, 128, NK*64]  layout [c][p][k*64 + r]
    zt = nc.dram_tensor("zt", [CPC, 128, NK * R], f16, kind="ExternalInput")
    # wt: [2, 128, CPC, NM, 128]  layout [conv][p][c][m][i]
    wt = nc.dram_tensor("wt", [2, 128, CPC, NM, 128], f16, kind="ExternalInput")
    # out: [2, 2048, CPC, 64]  layout [conv][t][c][r], fp8 e3m4 (x16 prescale)
    out = nc.dram_tensor("out", [2, S, CPC, R], f8, kind="ExternalOutput")

    ZHALF = GROUP // 2  # channels per z DMA slice
    N_WARMUP = 24       # dummy matmuls to bridge startup DMA + PE pstate ramp

    with TileContext(nc) as tc:
        with (
            tc.tile_pool(name="wupool", bufs=1) as wupool,
            tc.tile_pool(name="zpool", bufs=3) as zpool,
            tc.tile_pool(name="wpool", bufs=6) as wpool,
            tc.tile_pool(name="opool", bufs=3) as opool,
            tc.tile_pool(name="pspool", bufs=8, space="PSUM") as pspool,
        ):
            # PE warmup: dummy matmuls on a memset scratch tile keep the PE
            # continuously busy (and pstate-warm) while group 0's DMAs land.
            wu = wupool.tile([128, 640], f16, tag="wu")
            nc.vector.memset(wu[:], 0.0)
            for _ in range(N_WARMUP):
                ps = pspool.tile([128, 512], f32, tag="ps")
                nc.tensor.matmul(
                    ps[:], wu[:, 0:128], wu[:, 128:640], start=True, stop=True,
                )

            def w_dma(wtile, conv, c0, cw0, ncw):
                """DMA channels [c0+cw0, c0+cw0+ncw) of conv's weights into
                the matching slice of wtile."""
                wsrc = bass.AP(
                    wt,
                    conv * 128 * CPC * NM * 128 + (c0 + cw0) * NM * 128,
                    [[CPC * NM * 128, 128], [1, ncw * NM * 128]],
                )
                nc.sync.dma_start(
                    wtile[:, cw0 * NM * 128:(cw0 + ncw) * NM * 128], wsrc
                )

            def z_dma(ztile, c0, cz0, ncz):
                """DMA channels [c0+cz0, c0+cz0+ncz) of z into ztile."""
                src = bass.AP(
                    zt,
                    (c0 + cz0) * 128 * NK * R,
                    [[NK * R, 128], [128 * NK * R, ncz], [1, NK * R]],
                )
                nc.sync.dma_start(
                    ztile[:, cz0 * NK * R:(cz0 + ncz) * NK * R], src
                )

            evict_ctr = 0
            for gi in range(NGROUPS):
                c0 = gi * GROUP
                wtile0 = wpool.tile([128, GROUP * NM * 128], f16, tag="wt")
                ztile = zpool.tile([128, GROUP * NK * R], f16, tag="zt")
                wtile1 = wpool.tile([128, GROUP * NM * 128], f16, tag="wt")
                if gi == 0:
                    # Prologue: fine-grained interleave matching consumption
                    # order so the first matmuls unblock after ~2 small DMAs
                    # and arrivals keep pace with the conv0 channel sweep.
                    for cz in range(0, GROUP, 2):
                        z_dma(ztile, c0, cz, 2)
                        w_dma(wtile0, 0, c0, cz, 2)
                    w_dma(wtile1, 1, c0, 0, GROUP)
                else:
                    # Issue order for minimal first-matmul latency: w0, z, w1
                    w_dma(wtile0, 0, c0, 0, GROUP)
                    z_dma(ztile, c0, 0, ZHALF)
                    z_dma(ztile, c0, ZHALF, ZHALF)
                    w_dma(wtile1, 1, c0, 0, GROUP)
                wtiles = [wtile0, wtile1]

                for conv in range(2):
                    wtile = wtiles[conv]
                    # outbuf free layout: (T, c2, r) -> contiguous 512 B runs in DRAM
                    outbuf = opool.tile([128, NT * GROUP * R], f8, tag="ob")
                    ob4 = outbuf[:].rearrange(
                        "p (T c r) -> p T c r", T=NT, c=GROUP, r=R
                    )

                    def psum_group(h, c2, split_evict=False):
                        nonlocal evict_ctr
                        ps = pspool.tile([128, 512], f32, tag="ps")
                        for m in range(NM):
                            lhsT = wtile[:, (c2 * NM + m) * 128:(c2 * NM + m + 1) * 128]
                            rhs = ztile[:, c2 * NK * R + (m + 8 * h) * R:
                                        c2 * NK * R + (m + 8 * h) * R + 512]
                            nc.tensor.matmul(
                                ps[:], lhsT, rhs,
                                start=(m == 0), stop=(m == NM - 1),
                            )
                        # Evict PSUM -> outbuf slice (strided dest, f32->fp8)
                        psrc = ps[:].rearrange("p (T r) -> p T r", T=8, r=R)
                        if split_evict:
                            # Final group: halve latency by running both
                            # engines in parallel on T-halves.
                            nc.vector.tensor_copy(
                                ob4[:, 8 * h:8 * h + 4, c2, :], psrc[:, 0:4, :]
                            )
                            nc.scalar.copy(
                                ob4[:, 8 * h + 4:8 * h + 8, c2, :], psrc[:, 4:8, :]
                            )
                        elif evict_ctr % 2 == 0:
                            nc.vector.tensor_copy(ob4[:, 8 * h:8 * h + 8, c2, :], psrc)
                        else:
                            nc.scalar.copy(ob4[:, 8 * h:8 * h + 8, c2, :], psrc)
                        evict_ctr += 1

                    def out_dma(h, ck0, cpk, eng=None):
                        # Out DMA for channels [ck0, ck0+cpk) of T-range
                        # 8h..8h+7; (c, r)-contiguous runs. Issuing per-h
                        # halves the end-of-kernel DMA tail.
                        odst = bass.AP(
                            out,
                            conv * S * CPC * R + (1024 * h) * CPC * R
                            + (c0 + ck0) * R,
                            [[CPC * R, 128], [128 * CPC * R, NT // 2],
                             [1, cpk * R]],
                        )
                        (eng or nc.scalar).dma_start(
                            odst,
                            ob4[:, 8 * h:8 * h + 8, ck0:ck0 + cpk, :],
                        )

                    if gi == 0 and conv == 0:
                        # Prologue order: per-channel (both h) to match the
                        # fine-grained z/w DMA arrival cadence.
                        for c2 in range(GROUP):
                            for h in range(2):
                                psum_group(h, c2)
                        for h in range(2):
                            out_dma(h, 0, GROUP)
                    else:
                        last = (gi == NGROUPS - 1) and (conv == 1)
                        for h in range(2):
                            if last and h == 1:
                                # Epilogue: 2-channel chunks issued from the
                                # idle SP sequencer, interleaved between psum
                                # groups so only the final chunk's issue +
                                # descriptor-gen + transfer trail the last
                                # eviction.
                                for c2 in range(GROUP):
                                    psum_group(h, c2)
                                    if c2 in (1, 3, 5):
                                        out_dma(h, c2 - 1, 2, eng=nc.sync)
                                    elif c2 >= 6:
                                        out_dma(h, c2, 1, eng=nc.sync)
                            else:
                                for c2 in range(GROUP):
                                    psum_group(h, c2)
                                out_dma(h, 0, GROUP)

    _split_sync_waits(nc)
    return nc


_NC_CACHE = None


def kernel(z: np.ndarray, w_q: np.ndarray, w_k: np.ndarray):
    global _NC_CACHE

    # ---- Host-side prep -------------------------------------------------
    # z slice and transpose: zt[c, p, k, r] = z[r, 201 + 128k + p, c]
    zz = np.ascontiguousarray(z[:, 201:201 + NK * 128, :]).astype(np.float16)
    zz = zz.reshape(R, NK, 128, C)                     # [r, k, p, c]
    zt = np.ascontiguousarray(zz.transpose(3, 2, 1, 0))  # [c, p, k, r]
    zt = zt.reshape(NCORES, CPC, 128, NK * R)

    # Toeplitz blocks: W[m, p, i, c] = w[128m + p - i, 0, c] * OUT_PRESCALE/SCALE
    p = np.arange(128)[:, None]
    i = np.arange(128)[None, :]
    toep_list = []
    for w in (w_k, w_q):   # out[0] = conv with w_k (qbar), out[1] = conv with w_q (kbar)
        w = np.asarray(w, dtype=np.float32)
        blocks = np.zeros((NM, 128, 128, C), dtype=np.float32)  # fp32 build, fp16 ship
        for m in range(NM):
            J = 128 * m + p - i
            valid = (J >= 0) & (J < K)
            Jc = np.clip(J, 0, K - 1)
            blocks[m] = np.where(valid[:, :, None], w[Jc, 0, :], 0.0)
        blocks *= OUT_PRESCALE / SCALE
        blocks = blocks.astype(np.float16)
        # [m, p, i, c] -> [p, c, m, i] -> [p, cores, CPC, m, i]
        bt = np.ascontiguousarray(blocks.transpose(1, 3, 0, 2))
        toep_list.append(bt.reshape(128, NCORES, CPC, NM, 128))
    # wt per core: [2, 128, CPC, NM, 128]
    wts = [np.ascontiguousarray(np.stack([toep_list[0][:, g], toep_list[1][:, g]]))
           for g in range(NCORES)]

    in_maps = [{"zt": np.ascontiguousarray(zt[g]), "wt": wts[g]}
               for g in range(NCORES)]

    # ---- Build + run ----------------------------------------------------
    if _NC_CACHE is None:
        _NC_CACHE = _build_nc()
    import os
    trace = bool(int(os.environ.get("KERNEL_TRACE", "0")))
    res = run_bass_kernel_spmd(
        _NC_CACHE, in_maps, core_ids=list(range(NCORES)), trace=trace,
    )
    kernel.last_result = res

    # ---- Gather ---------------------------------------------------------
    # Reference applies a RAW row-major reshape [R, S*C] -> [R, H, F, S'] then
    # transpose, so: out[conv][0, s, h, f, r] = conv[r, 256h + 4f + s//512, s % 512].
    arr = np.stack([res.results[g]["out"] for g in range(NCORES)]).astype(np.float32)
    arr *= 1.0 / OUT_PRESCALE
    # arr: [g, conv, t, c_local, r] -> conv_all[conv, t, c, r]
    conv_all = arr.transpose(1, 2, 0, 3, 4).reshape(2, S, C, R)
    # t = 256h + 4f + a  (row-major h, f, a); s = 512a + c
    x = conv_all.reshape(2, H, F, 4, C, R)            # [conv, h, f, a, c, r]
    x = x.transpose(0, 3, 4, 1, 2, 5).reshape(2, S, H, F, R)
    q = np.ascontiguousarray(x[0])[None]
    kk = np.ascontiguousarray(x[1])[None]
    return q, kk


# revision 27
# speedup vs baseline: 1.0011x; 1.0011x over previous
"""Trainium2 Bass kernel for nn_ConvSPE (two depthwise convs K=201 over z).

Strategy
--------
out[t, c] = sum_j w[j, c] * z[201 + t + j, c]   (t in [0, 2048), per realization r)

Mapped to dense PE matmuls via banded-Toeplitz weight blocks: for output tile
t = 128*T + i, the contraction (i + j) splits into 3 chunks of 128 (m = 0..2):

    out[128T + i] = sum_m sum_p  W_m[p, i] * z[201 + 128(T+m) + p]
    W_m[p, i] = w[128m + p - i]   (zero outside [0, 201))

W_m is independent of T and r, so one stationary [128, 128] weight block
streams all 64 realizations x all 16 output tiles as matmul columns.

Sharding: channels across the 8 cores (64 ch = one head per core); weights and
z-slices per channel are core-private, realizations all stay on-core.

dtypes: fp16 matmul inputs (rel err ~3e-4) accumulated in f32 PSUM; outputs
quantized to fp8 e3m4 with a x16 pre-scale folded into the weights (rel err
~1.3%, inside the 2e-2 budget) to halve output HBM traffic.

DMA layout notes: wt is shipped as [conv][p][c][m][i] so each (group, conv)
weight DMA reads 6 KiB contiguous runs per partition (128 descriptors); the
z DMA ships [c][p][k*64+r] with 2.25 KiB runs; out DMA writes (c, r)-contiguous
512 B runs per (tile, partition).

Schedule: dummy warmup matmuls bridge the startup DMA latency and the PE
p-state ramp; group 0 streams fine-grained per-channel z/w DMAs interleaved
with a channel-major matmul order so compute starts ~4.3 us in; the final
(group, conv, h) block issues tapered per-channel output DMAs from the idle
SP queue between PSUM evictions to minimize the end-of-kernel tail.
"""

import numpy as np
import concourse.bass as bass
import concourse.mybir as mybir
from concourse.tile import TileContext
from concourse.bass_utils import run_bass_kernel_spmd

# Problem constants (hardcoded per the task contract)
R = 64
S = 2048
K = 201
C = 512
H = 8
F = 64
PAD_LEN = 4 * K + S  # 2852
SCALE = float((R * F) ** 0.25)  # 8.0
OUT_PRESCALE = 16.0  # folded into weights; host divides it back out

NCORES = 8
CPC = C // NCORES      # 64 channels per core
NK = 18                # 128-element z chunks per channel: u in [201, 201 + 18*128)
NT = S // 128          # 16 output tiles
NM = 3                 # Toeplitz chunks per output tile
GROUP = 8              # channels processed per DMA group
NGROUPS = CPC // GROUP


def _split_sync_waits(nc) -> None:
    """Walrus in this container accepts at most ONE sync wait per instruction.

    Move extra on_wait entries onto same-engine InstNoOp carriers inserted
    immediately before the over-limit instruction (program order on the same
    engine preserves semantics)."""
    ctr = 0
    for f in nc.m.functions:
        for blk in f.blocks:
            new = []
            for inst in blk.instructions:
                si = inst.sync_info
                waits = list(si.on_wait) if (si is not None and si.on_wait) else []
                if len(waits) > 1:
                    for wjob in waits[:-1]:
                        nop = mybir.InstNoOp(name=f"antwaitnop{ctr}", ins=[], outs=[])
                        ctr += 1
                        nop.engine = inst.engine
                        nop.sync_info = mybir.SyncInfo(on_wait=[wjob], on_update=[])
                        new.append(nop)
                    si.on_wait = [waits[-1]]
                new.append(inst)
            blk.instructions = new


def _build_nc():
    """Build the per-core Bass program (identical on all 8 cores)."""
    nc = bass.Bass()
    f32 = mybir.dt.float32
    f16 = mybir.dt.float16
    f8 = mybir.dt.float8e3

    # zt: [CPC, 128, NK*64]  layout [c][p][k*64 + r]
    zt = nc.dram_tensor("zt", [CPC, 128, NK * R], f16, kind="ExternalInput")
    # wt: [2, 128, CPC, NM, 128]  layout [conv][p][c][m][i]
    wt = nc.dram_tensor("wt", [2, 128, CPC, NM, 128], f16, kind="ExternalInput")
    # out: [2, 2048, CPC, 64]  layout [conv][t][c][r], fp8 e3m4 (x16 prescale)
    out = nc.dram_tensor("out", [2, S, CPC, R], f8, kind="ExternalOutput")

    ZHALF = GROUP // 2  # channels per z DMA slice
    N_WARMUP = 5       # dummy matmuls to bridge startup DMA + PE pstate ramp

    with TileContext(nc) as tc:
        with (
            tc.tile_pool(name="wupool", bufs=1) as wupool,
            tc.tile_pool(name="zpool", bufs=3) as zpool,
            tc.tile_pool(name="wpool", bufs=6) as wpool,
            tc.tile_pool(name="opool", bufs=3) as opool,
            tc.tile_pool(name="pspool", bufs=8, space="PSUM") as pspool,
        ):
            # PE warmup: dummy matmuls on a memset scratch tile keep the PE
            # continuously busy (and pstate-warm) while group 0's DMAs land.
            wu = wupool.tile([128, 640], f16, tag="wu")
            nc.vector.memset(wu[:], 0.0)
            for _ in range(N_WARMUP):
                ps = pspool.tile([128, 512], f32, tag="ps")
                nc.tensor.matmul(
                    ps[:], wu[:, 0:128], wu[:, 128:640], start=True, stop=True,
                )

            def w_dma(wtile, conv, c0, cw0, ncw):
                """DMA channels [c0+cw0, c0+cw0+ncw) of conv's weights into
                the matching slice of wtile."""
                wsrc = bass.AP(
                    wt,
                    conv * 128 * CPC * NM * 128 + (c0 + cw0) * NM * 128,
                    [[CPC * NM * 128, 128], [1, ncw * NM * 128]],
                )
                nc.sync.dma_start(
                    wtile[:, cw0 * NM * 128:(cw0 + ncw) * NM * 128], wsrc
                )

            def z_dma(ztile, c0, cz0, ncz):
                """DMA channels [c0+cz0, c0+cz0+ncz) of z into ztile."""
                src = bass.AP(
                    zt,
                    (c0 + cz0) * 128 * NK * R,
                    [[NK * R, 128], [128 * NK * R, ncz], [1, NK * R]],
                )
                nc.sync.dma_start(
                    ztile[:, cz0 * NK * R:(cz0 + ncz) * NK * R], src
                )

            evict_ctr = 1  # start parity: ACT first (scheduling-tuned)
            for gi in range(NGROUPS):
                c0 = gi * GROUP
                wtile0 = wpool.tile([128, GROUP * NM * 128], f16, tag="wt")
                ztile = zpool.tile([128, GROUP * NK * R], f16, tag="zt")
                wtile1 = wpool.tile([128, GROUP * NM * 128], f16, tag="wt")
                if gi == 0:
                    # Prologue: fine-grained interleave matching consumption
                    # order so the first matmuls unblock after ~2 small DMAs
                    # and arrivals keep pace with the conv0 channel sweep.
                    for cz1 in range(5):
                        z_dma(ztile, c0, cz1, 1)
                        w_dma(wtile0, 0, c0, cz1, 1)
                    z_dma(ztile, c0, 5, 2)
                    w_dma(wtile0, 0, c0, 5, 2)
                    z_dma(ztile, c0, 7, 1)
                    w_dma(wtile0, 0, c0, 7, 1)
                    w_dma(wtile1, 1, c0, 0, ZHALF)
                    w_dma(wtile1, 1, c0, ZHALF, ZHALF)
                else:
                    # Issue order for minimal first-matmul latency: w0, z, w1
                    w_dma(wtile0, 0, c0, 0, GROUP)
                    z_dma(ztile, c0, 0, ZHALF)
                    z_dma(ztile, c0, ZHALF, ZHALF)
                    w_dma(wtile1, 1, c0, 0, GROUP)
                wtiles = [wtile0, wtile1]

                for conv in range(2):
                    wtile = wtiles[conv]
                    # outbuf free layout: (T, c2, r) -> contiguous 512 B runs in DRAM
                    outbuf = opool.tile([128, NT * GROUP * R], f8, tag="ob")
                    ob4 = outbuf[:].rearrange(
                        "p (T c r) -> p T c r", T=NT, c=GROUP, r=R
                    )

                    def psum_group(h, c2, split_evict=False):
                        nonlocal evict_ctr
                        ps = pspool.tile([128, 512], f32, tag="ps")
                        for m in range(NM):
                            lhsT = wtile[:, (c2 * NM + m) * 128:(c2 * NM + m + 1) * 128]
                            rhs = ztile[:, c2 * NK * R + (m + 8 * h) * R:
                                        c2 * NK * R + (m + 8 * h) * R + 512]
                            nc.tensor.matmul(
                                ps[:], lhsT, rhs,
                                start=(m == 0), stop=(m == NM - 1),
                            )
                        # Evict PSUM -> outbuf slice (strided dest, f32->fp8)
                        psrc = ps[:].rearrange("p (T r) -> p T r", T=8, r=R)
                        if split_evict:
                            # Final group: halve latency by running both
                            # engines in parallel on T-halves.
                            nc.vector.tensor_copy(
                                ob4[:, 8 * h:8 * h + 4, c2, :], psrc[:, 0:4, :]
                            )
                            nc.scalar.copy(
                                ob4[:, 8 * h + 4:8 * h + 8, c2, :], psrc[:, 4:8, :]
                            )
                        elif evict_ctr % 2 == 0:
                            nc.vector.tensor_copy(ob4[:, 8 * h:8 * h + 8, c2, :], psrc)
                        else:
                            nc.scalar.copy(ob4[:, 8 * h:8 * h + 8, c2, :], psrc)
                        evict_ctr += 1

                    def out_dma(h, ck0, cpk, eng=None):
                        # Out DMA for channels [ck0, ck0+cpk) of T-range
                        # 8h..8h+7; (c, r)-contiguous runs. Issuing per-h
                        # halves the end-of-kernel DMA tail.
                        odst = bass.AP(
                            out,
                            conv * S * CPC * R + (1024 * h) * CPC * R
                            + (c0 + ck0) * R,
                            [[CPC * R, 128], [128 * CPC * R, NT // 2],
                             [1, cpk * R]],
                        )
                        (eng or nc.scalar).dma_start(
                            odst,
                            ob4[:, 8 * h:8 * h + 8, ck0:ck0 + cpk, :],
                        )

                    if gi == 0 and conv == 0:
                        # Prologue order: per-channel (both h) to match the
                        # fine-grained z/w DMA arrival cadence.
                        for c2 in range(GROUP):
                            for h in range(2):
                                psum_group(h, c2)
                        for h in range(2):
                            out_dma(h, 0, GROUP)
                    else:
                        last = (gi == NGROUPS - 1) and (conv == 1)
                        for h in range(2):
                            if last and h == 1:
                                # Epilogue: 2-channel chunks issued from the
                                # idle SP sequencer, interleaved between psum
                                # groups so only the final chunk's issue +
                                # descriptor-gen + transfer trail the last
                                # eviction.
                                for c2 in range(GROUP):
                                    psum_group(h, c2)
                                    if c2 in (1, 3, 5):
                                        out_dma(h, c2 - 1, 2, eng=nc.sync)
                                    elif c2 >= 6:
                                        out_dma(h, c2, 1, eng=nc.sync)
                            else:
                                for c2 in range(GROUP):
                                    psum_group(h, c2)
                                out_dma(h, 0, GROUP)

    _split_sync_waits(nc)
    return nc


_NC_CACHE = None


def kernel(z: np.ndarray, w_q: np.ndarray, w_k: np.ndarray):
    global _NC_CACHE

    # ---- Host-side prep -------------------------------------------------
    # z slice and transpose: zt[c, p, k, r] = z[r, 201 + 128k + p, c]
    zz = np.ascontiguousarray(z[:, 201:201 + NK * 128, :]).astype(np.float16)
    zz = zz.reshape(R, NK, 128, C)                     # [r, k, p, c]
    zt = np.ascontiguousarray(zz.transpose(3, 2, 1, 0))  # [c, p, k, r]
    zt = zt.reshape(NCORES, CPC, 128, NK * R)

    # Toeplitz blocks: W[m, p, i, c] = w[128m + p - i, 0, c] * OUT_PRESCALE/SCALE
    p = np.arange(128)[:, None]
    i = np.arange(128)[None, :]
    toep_list = []
    for w in (w_k, w_q):   # out[0] = conv with w_k (qbar), out[1] = conv with w_q (kbar)
        w = np.asarray(w, dtype=np.float32)
        blocks = np.zeros((NM, 128, 128, C), dtype=np.float32)  # fp32 build, fp16 ship
        for m in range(NM):
            J = 128 * m + p - i
            valid = (J >= 0) & (J < K)
            Jc = np.clip(J, 0, K - 1)
            blocks[m] = np.where(valid[:, :, None], w[Jc, 0, :], 0.0)
        blocks *= OUT_PRESCALE / SCALE
        blocks = blocks.astype(np.float16)
        # [m, p, i, c] -> [p, c, m, i] -> [p, cores, CPC, m, i]
        bt = np.ascontiguousarray(blocks.transpose(1, 3, 0, 2))
        toep_list.append(bt.reshape(128, NCORES, CPC, NM, 128))
    # wt per core: [2, 128, CPC, NM, 128]
    wts = [np.ascontiguousarray(np.stack([toep_list[0][:, g], toep_list[1][:, g]]))
           for g in range(NCORES)]

    in_maps = [{"zt": np.ascontiguousarray(zt[g]), "wt": wts[g]}
               for g in range(NCORES)]

    # ---- Build + run ----------------------------------------------------
    if _NC_CACHE is None:
        _NC_CACHE = _build_nc()
    import os
    trace = bool(int(os.environ.get("KERNEL_TRACE", "0")))
    res = run_bass_kernel_spmd(
        _NC_CACHE, in_maps, core_ids=list(range(NCORES)), trace=trace,
    )
    kernel.last_result = res

    # ---- Gather ---------------------------------------------------------
    # Reference applies a RAW row-major reshape [R, S*C] -> [R, H, F, S'] then
    # transpose, so: out[conv][0, s, h, f, r] = conv[r, 256h + 4f + s//512, s % 512].
    arr = np.stack([res.results[g]["out"] for g in range(NCORES)]).astype(np.float32)
    arr *= 1.0 / OUT_PRESCALE
    # arr: [g, conv, t, c_local, r] -> conv_all[conv, t, c, r]
    conv_all = arr.transpose(1, 2, 0, 3, 4).reshape(2, S, C, R)
    # t = 256h + 4f + a  (row-major h, f, a); s = 512a + c
    x = conv_all.reshape(2, H, F, 4, C, R)            # [conv, h, f, a, c, r]
    x = x.transpose(0, 3, 4, 1, 2, 5).reshape(2, S, H, F, R)
    q = np.ascontiguousarray(x[0])[None]
    kk = np.ascontiguousarray(x[1])[None]
    return q, kk

